# revision 21
# baseline (speedup 1.0000x reference)
"""Trainium2 Bass kernel for nn_Attention_11192684774063.

Attention over channels: for each batch b (C=256 channels, N=2048 positions)
    scores = q^T k / sqrt(N)        # [N, N], contraction over C
    p      = softmax(scores, -1)    # [N, N]
    out    = v @ p^T                # [C, N], contraction over N (keys)
Returns (out [B,C,N], p_attn [B,N,N]) like the reference.

Sharding: B == 8 == n_cores -> one batch per NeuronCore, pure data parallel
(no collectives). Each core computes p_attn[b] and out[b]^T; the host stacks
the per-core results and transposes out back to [C, N].

Device program notes:
 - Matmuls run in float32r (full-rate fp32 PE mode; fp32 accumulate in PSUM).
 - softmax skips the max-subtraction: scores here are ~N(0, 0.35), |s| < ~2,
   so exp is safe in fp32 and matches jax.nn.softmax to fp32 accuracy.
 - exp runs on the scalar engine with the 1/sqrt(N) scale folded in;
   accum_out produces the softmax row sums for free.
 - The out matmul needs key-position (m) on the partition axis, so scores
   are computed a second time transposed (k^T q) instead of transposing the
   16MB p matrix through the PE.
 - The natural-scores pipeline (ACT-heavy) and the transposed-scores ->
   out^T pipeline (PE-heavy) are interleaved unit-by-unit into one software
   pipeline so both engines stay busy; PSUM is split 4 banks (natural
   scores) + 2 (transposed scores) + 2 (out accumulators).
 - The 4 out accumulator chains per n-quarter live packed 2-per-bank; since
   a start=True matmul clears has_written for its whole bank, each bank is
   instead initialized once by a dummy zero matmul and the chains accumulate
   with start=False.
 - Input DMAs are split and ordered by first use (q half, k, v, q half) so
   the PE starts ~6us in instead of after the full 6MB load.
"""

import sys

if "/opt/trn_rl_repo" not in sys.path:
    sys.path.insert(0, "/opt/trn_rl_repo")

import math

import numpy as np

import concourse.bass as bass  # noqa: F401
import concourse.mybir as mybir
import concourse.tile as tile
from concourse import bacc
from concourse.bass_utils import run_bass_kernel_spmd
from concourse.masks import make_identity

B, C, N = 8, 256, 2048
P = 128  # partitions
CT = C // P  # 2 c-chunks
NT = N // P  # 16 n/m tiles
SCALE = 1.0 / math.sqrt(float(N))

F32 = mybir.dt.float32
F32R = mybir.dt.float32r

NU = 2 * NT  # 32 pipeline units (one B-step each; one A-block per 2 units)
LAG = 8  # units between e^T production and its consumption by out matmuls


def build_program():
    nc = bacc.Bacc("TRN2", target_bir_lowering=False, debug=False)

    q_d = nc.dram_tensor("q", [C, N], F32, kind="ExternalInput")
    k_d = nc.dram_tensor("k", [C, N], F32, kind="ExternalInput")
    v_d = nc.dram_tensor("v", [C, N], F32, kind="ExternalInput")
    p_d = nc.dram_tensor("p", [N, N], F32, kind="ExternalOutput")
    ot_d = nc.dram_tensor("ot", [N, C], F32, kind="ExternalOutput")  # out^T

    Exp = mybir.ActivationFunctionType.Exp
    H2 = N // 2

    with tile.TileContext(nc) as tc:
        with (
            tc.tile_pool(name="singles", bufs=1) as singles,
            tc.tile_pool(name="ea", bufs=6) as ea_pool,
            tc.tile_pool(name="etb", bufs=LAG + 4) as et_pool,
            tc.tile_pool(name="osb", bufs=6) as osb_pool,
        ):
            # ---- persistent SBUF + staged input DMAs -------------------
            qs = [
                singles.tile([P, N], F32R, tag=f"q{c}", name=f"q{c}")
                for c in range(CT)
            ]
            ks = [
                singles.tile([P, N], F32R, tag=f"k{c}", name=f"k{c}")
                for c in range(CT)
            ]
            vs = [
                singles.tile([P, N], F32, tag=f"v{c}", name=f"v{c}")
                for c in range(CT)
            ]

            def dma_in(eng, dst, src_dram, c, lo, hi, cast=None):
                src = src_dram[c * P : (c + 1) * P, lo:hi]
                if cast is not None:
                    src = src.bitcast(cast)
                eng.dma_start(dst[:, lo:hi], src)

            # one HWDGE FIFO = strict priority, halves ordered by first
            # use: q h0 (stB rhs + early A lhsT), k h0 (stB lhsT), k h1
            # (A rhs tail), v (transposes at units 4..7), q h1 (A(8+))
            for c in range(CT):
                dma_in(nc.sync, qs[c], q_d, c, 0, H2, F32R)
            for c in range(CT):
                dma_in(nc.sync, ks[c], k_d, c, 0, H2, F32R)
            for c in range(CT):
                dma_in(nc.sync, ks[c], k_d, c, H2, N, F32R)
            for c in range(CT):
                dma_in(nc.sync, vs[c], v_d, c, 0, H2)
            for c in range(CT):
                dma_in(nc.sync, vs[c], v_d, c, H2, N)
            for c in range(CT):
                dma_in(nc.sync, qs[c], q_d, c, H2, N, F32R)

            ident = singles.tile([P, P], F32, tag="ident", name="ident")
            make_identity(nc, ident[:])
            zeros_f = singles.tile([P, P], F32, tag="zeros_f", name="zeros_f")
            nc.vector.memset(zeros_f[:], 0.0)
            zeros = singles.tile([P, P], F32R, tag="zeros", name="zeros")
            nc.vector.tensor_copy(zeros[:], zeros_f[:])

            denom = singles.tile([P, NT], F32, tag="denom", name="denom")
            recip = singles.tile([P, NT], F32, tag="recip", name="recip")

            # v^T [m, c]: vt[:, mt, :] is the [128m, 256c] block for m-tile mt
            vt = singles.tile([P, NT, C], F32R, tag="vt", name="vt")

            def emit_stB(u):
                """Transposed-scores matmuls for pipeline unit u."""
                quarter, mmi = u // 8, u % 8
                n0 = quarter * 512
                st = psBs.tile([P, 2, 512], F32, tag="st", name="st")
                for j in range(2):
                    m = 2 * mmi + j
                    for c in range(CT):
                        nc.tensor.matmul(
                            st[:, j, :],
                            lhsT=ks[c][:, m * P : (m + 1) * P],
                            rhs=qs[c][:, n0 : n0 + 512],
                            start=(c == 0),
                            stop=(c == CT - 1),
                        )
                et = et_pool.tile([P, 2, 512], F32R, tag="et", name="et")
                nc.scalar.activation(et[:], st[:], Exp, scale=SCALE)
                return et

            def emit_A(i):
                """Natural-scores block i: matmuls, exp+sum, normalize, store."""
                s = psA.tile([P, N], F32, tag="s", name="s")
                for c in range(CT):
                    for mc in range(N // 512):
                        nc.tensor.matmul(
                            s[:, mc * 512 : (mc + 1) * 512],
                            lhsT=qs[c][:, i * P : (i + 1) * P],
                            rhs=ks[c][:, mc * 512 : (mc + 1) * 512],
                            start=(c == 0),
                            stop=(c == CT - 1),
                        )
                e = ea_pool.tile([P, N], F32, tag="e", name="e")
                nc.scalar.activation(
                    e[:], s[:], Exp, scale=SCALE, accum_out=denom[:, i : i + 1]
                )
                nc.vector.reciprocal(recip[:, i : i + 1], denom[:, i : i + 1])
                nc.vector.tensor_scalar_mul(e[:], e[:], recip[:, i : i + 1])
                nc.sync.dma_start(p_d[i * P : (i + 1) * P, :], e[:])

            def emit_vt(mt):
                """PE-transpose v m-tile mt into vt."""
                tp = psBo.tile([P, P], F32, tag="outp", name="tp")
                for c in range(CT):
                    nc.tensor.transpose(
                        tp[:], vs[c][:, mt * P : (mt + 1) * P], ident[:]
                    )
                    nc.vector.tensor_copy(vt[:, mt, c * P : (c + 1) * P], tp[:])

            def emit_outB(w, ets, outp):
                """Out^T accumulation consuming unit w's e^T tile."""
                mmi = w % 8
                et = ets[w]
                for j in range(2):
                    m = 2 * mmi + j
                    for nt in range(4):
                        nc.tensor.matmul(
                            outp[nt // 2][:, (nt % 2) * C : (nt % 2) * C + C],
                            lhsT=et[:, j, nt * P : (nt + 1) * P],
                            rhs=vt[:, m, :],
                            start=False,
                            stop=(m == NT - 1),
                            skip_group_check=True,
                        )

            def emit_flush(quarter, outp):
                """Normalize + store the 4 finished n-tiles of a quarter."""
                for nt in range(4):
                    g = quarter * 4 + nt
                    osb = osb_pool.tile([P, C], F32, tag="osb", name="osb")
                    nc.vector.tensor_scalar_mul(
                        osb[:],
                        outp[nt // 2][:, (nt % 2) * C : (nt % 2) * C + C],
                        recip[:, g : g + 1],
                    )
                    nc.scalar.dma_start(ot_d[g * P : (g + 1) * P, :], osb[:])

            ets = {}
            with (
                tc.tile_pool(name="psBs", bufs=1, space="PSUM") as psBs,
                tc.tile_pool(name="psBo", bufs=2, space="PSUM") as psBo,
            ):
                # psA opens AFTER psBo so it tops the pool stack and can be
                # released (LIFO) before the pipeline tail needs its banks
                psA_cm = tc.tile_pool(name="psA", bufs=1, space="PSUM")
                psA = psA_cm.__enter__()
                # A-block schedule: one per even unit, then densified at
                # units 24..27 so psA can close before the pipeline tail
                A_at = {2 * i: i for i in range(12)}
                for i in range(12, NT):
                    A_at[12 + i] = i

                # units 0..7: quarter 0 front + v transposes (the transpose
                # scratch shares psBo's bank-padded slots via tag="outp")
                for u in range(LAG):
                    ets[u] = emit_stB(u)
                    if u in A_at:
                        emit_A(A_at[u])
                    if u >= 4:
                        for mt in range(4 * (u - 4), 4 * (u - 3)):
                            emit_vt(mt)

                def alloc_outp(pool):
                    outp = [
                        pool.tile([P, 512], F32, tag="outp", name="outp")
                        for _ in range(2)
                    ]
                    for t in outp:
                        nc.tensor.matmul(
                            t[:],
                            lhsT=zeros[:],
                            rhs=ks[0][:, 0:512],
                            start=True,
                            stop=False,
                            skip_group_check=True,
                        )
                    return outp

                outp = None
                for u in range(LAG, 28):
                    w = u - LAG  # 0..19: quarters 0..2 at lag 8
                    if w % 8 == 0:
                        outp = alloc_outp(psBo)
                    ets[u] = emit_stB(u)
                    if u in A_at:
                        emit_A(A_at[u])
                    emit_outB(w, ets, outp)
                    del ets[w]
                    if w % 8 == 7:
                        emit_flush(w // 8, outp)
                # all 16 A blocks emitted; free psA's 4 banks for the tail
                psA_cm.__exit__(None, None, None)

                with tc.tile_pool(name="psBo2", bufs=2, space="PSUM") as psBo2:
                    outp3 = None
                    for u in range(28, 36):
                        if u == 28:
                            outp3 = alloc_outp(psBo2)
                        if u < NU:
                            ets[u] = emit_stB(u)
                        if u <= 31:
                            # quarter 2 finishes at lag 8
                            w = u - LAG
                            emit_outB(w, ets, outp)
                            del ets[w]
                            if w % 8 == 7:
                                emit_flush(2, outp)
                        # quarter 3 runs at lag 4
                        w3 = u - 4
                        emit_outB(w3, ets, outp3)
                        del ets[w3]
                        if w3 == NU - 1:
                            emit_flush(3, outp3)

    nc.compile()
    return nc


_NC_CACHE = None


def _get_program():
    global _NC_CACHE
    if _NC_CACHE is None:
        _NC_CACHE = build_program()
    return _NC_CACHE


def kernel(query, key, value, mask=0, _collect=None):
    """Full inputs in, full outputs out. Shards batch across 8 cores."""
    query = np.ascontiguousarray(np.asarray(query, dtype=np.float32))
    key = np.ascontiguousarray(np.asarray(key, dtype=np.float32))
    value = np.ascontiguousarray(np.asarray(value, dtype=np.float32))
    assert query.shape == (B, C, N), query.shape

    nc = _get_program()
    in_maps = [
        {"q": query[b], "k": key[b], "v": value[b]} for b in range(B)
    ]
    res = run_bass_kernel_spmd(nc, in_maps, core_ids=list(range(B)))
    if _collect is not None:
        _collect.append(res)
    p_attn = np.stack([res.results[b]["p"] for b in range(B)])
    out = np.stack([res.results[b]["ot"].T for b in range(B)])
    return out, p_attn


# revision 22
# speedup vs baseline: 1.0294x; 1.0294x over previous
"""Trainium2 Bass kernel for nn_Attention_11192684774063.

Attention over channels: for each batch b (C=256 channels, N=2048 positions)
    scores = q^T k / sqrt(N)        # [N, N], contraction over C
    p      = softmax(scores, -1)    # [N, N]
    out    = v @ p^T                # [C, N], contraction over N (keys)
Returns (out [B,C,N], p_attn [B,N,N]) like the reference.

Sharding: B == 8 == n_cores -> one batch per NeuronCore, pure data parallel
(no collectives). Each core computes p_attn[b] and out[b]^T; the host stacks
the per-core results and transposes out back to [C, N].

Device program notes:
 - Matmuls run in float32r (full-rate fp32 PE mode; fp32 accumulate in PSUM).
 - softmax skips the max-subtraction: scores here are ~N(0, 0.35), |s| < ~2,
   so exp is safe in fp32 and matches jax.nn.softmax to fp32 accuracy.
 - exp runs on the scalar engine with the 1/sqrt(N) scale folded in;
   accum_out produces the softmax row sums for free.
 - The out matmul needs key-position (m) on the partition axis, so scores
   are computed a second time transposed (k^T q) instead of transposing the
   16MB p matrix through the PE.
 - The natural-scores pipeline (ACT-heavy) and the transposed-scores ->
   out^T pipeline (PE-heavy) are interleaved unit-by-unit into one software
   pipeline so both engines stay busy; PSUM is split 4 banks (natural
   scores) + 2 (transposed scores) + 2 (out accumulators).
 - The 4 out accumulator chains per n-quarter live packed 2-per-bank; since
   a start=True matmul clears has_written for its whole bank, each bank is
   instead initialized once by a dummy zero matmul and the chains accumulate
   with start=False.
 - Input DMAs are split and ordered by first use (q half, k, v, q half) so
   the PE starts ~6us in instead of after the full 6MB load.
"""

import sys

if "/opt/trn_rl_repo" not in sys.path:
    sys.path.insert(0, "/opt/trn_rl_repo")

import math

import numpy as np

import concourse.bass as bass  # noqa: F401
import concourse.mybir as mybir
import concourse.tile as tile
from concourse import bacc
from concourse.bass_utils import run_bass_kernel_spmd
from concourse.masks import make_identity

B, C, N = 8, 256, 2048
P = 128  # partitions
CT = C // P  # 2 c-chunks
NT = N // P  # 16 n/m tiles
SCALE = 1.0 / math.sqrt(float(N))

F32 = mybir.dt.float32
F32R = mybir.dt.float32r

NU = 2 * NT  # 32 pipeline units (one B-step each; one A-block per 2 units)
LAG = 8  # units between e^T production and its consumption by out matmuls


def build_program():
    nc = bacc.Bacc("TRN2", target_bir_lowering=False, debug=False)

    q_d = nc.dram_tensor("q", [C, N], F32, kind="ExternalInput")
    k_d = nc.dram_tensor("k", [C, N], F32, kind="ExternalInput")
    v_d = nc.dram_tensor("v", [C, N], F32, kind="ExternalInput")
    p_d = nc.dram_tensor("p", [N, N], F32, kind="ExternalOutput")
    ot_d = nc.dram_tensor("ot", [N, C], F32, kind="ExternalOutput")  # out^T

    Exp = mybir.ActivationFunctionType.Exp
    H2 = N // 2

    with tile.TileContext(nc) as tc:
        with (
            tc.tile_pool(name="singles", bufs=1) as singles,
            tc.tile_pool(name="ea", bufs=6) as ea_pool,
            tc.tile_pool(name="etb", bufs=LAG + 4) as et_pool,
            tc.tile_pool(name="osb", bufs=6) as osb_pool,
        ):
            # ---- persistent SBUF + staged input DMAs -------------------
            qs = [
                singles.tile([P, N], F32R, tag=f"q{c}", name=f"q{c}")
                for c in range(CT)
            ]
            ks = [
                singles.tile([P, N], F32R, tag=f"k{c}", name=f"k{c}")
                for c in range(CT)
            ]
            vs = [
                singles.tile([P, N], F32, tag=f"v{c}", name=f"v{c}")
                for c in range(CT)
            ]

            def dma_in(eng, dst, src_dram, c, lo, hi, cast=None):
                src = src_dram[c * P : (c + 1) * P, lo:hi]
                if cast is not None:
                    src = src.bitcast(cast)
                eng.dma_start(dst[:, lo:hi], src)

            # one HWDGE FIFO = strict priority, halves ordered by first
            # use: q h0 (stB rhs + early A lhsT), k h0 (stB lhsT), k h1
            # (A rhs tail), v (transposes at units 4..7), q h1 (A(8+))
            for c in range(CT):
                dma_in(nc.sync, qs[c], q_d, c, 0, H2, F32R)
            for c in range(CT):
                dma_in(nc.sync, ks[c], k_d, c, 0, H2, F32R)
            for c in range(CT):
                dma_in(nc.sync, ks[c], k_d, c, H2, N, F32R)
            for c in range(CT):
                dma_in(nc.sync, vs[c], v_d, c, 0, H2)
            for c in range(CT):
                dma_in(nc.sync, vs[c], v_d, c, H2, N)
            for c in range(CT):
                dma_in(nc.sync, qs[c], q_d, c, H2, N, F32R)

            ident = singles.tile([P, P], F32, tag="ident", name="ident")
            make_identity(nc, ident[:])
            zeros_f = singles.tile([P, P], F32, tag="zeros_f", name="zeros_f")
            nc.vector.memset(zeros_f[:], 0.0)
            zeros = singles.tile([P, P], F32R, tag="zeros", name="zeros")
            nc.vector.tensor_copy(zeros[:], zeros_f[:])

            denom = singles.tile([P, NT], F32, tag="denom", name="denom")
            recip = singles.tile([P, NT], F32, tag="recip", name="recip")

            # v^T [m, c]: vt[:, mt, :] is the [128m, 256c] block for m-tile mt
            vt = singles.tile([P, NT, C], F32R, tag="vt", name="vt")

            def emit_stB(u):
                """Transposed-scores matmuls for pipeline unit u."""
                quarter, mmi = u // 8, u % 8
                n0 = quarter * 512
                st = psBs.tile([P, 2, 512], F32, tag="st", name="st")
                for j in range(2):
                    m = 2 * mmi + j
                    for c in range(CT):
                        nc.tensor.matmul(
                            st[:, j, :],
                            lhsT=ks[c][:, m * P : (m + 1) * P],
                            rhs=qs[c][:, n0 : n0 + 512],
                            start=(c == 0),
                            stop=(c == CT - 1),
                        )
                et = et_pool.tile([P, 2, 512], F32R, tag="et", name="et")
                nc.scalar.activation(et[:], st[:], Exp, scale=SCALE)
                return et

            def emit_A(i):
                """Natural-scores block i: matmuls, exp+sum, normalize, store."""
                s = psA.tile([P, N], F32, tag="s", name="s")
                for c in range(CT):
                    for mc in range(N // 512):
                        nc.tensor.matmul(
                            s[:, mc * 512 : (mc + 1) * 512],
                            lhsT=qs[c][:, i * P : (i + 1) * P],
                            rhs=ks[c][:, mc * 512 : (mc + 1) * 512],
                            start=(c == 0),
                            stop=(c == CT - 1),
                        )
                e = ea_pool.tile([P, N], F32, tag="e", name="e")
                nc.scalar.activation(
                    e[:], s[:], Exp, scale=SCALE, accum_out=denom[:, i : i + 1]
                )
                nc.vector.reciprocal(recip[:, i : i + 1], denom[:, i : i + 1])
                nc.vector.tensor_scalar_mul(e[:], e[:], recip[:, i : i + 1])
                nc.sync.dma_start(p_d[i * P : (i + 1) * P, :], e[:])

            def emit_vt(mt):
                """PE-transpose v m-tile mt into vt."""
                tp = psBo.tile([P, P], F32, tag="outp", name="tp")
                for c in range(CT):
                    nc.tensor.transpose(
                        tp[:], vs[c][:, mt * P : (mt + 1) * P], ident[:]
                    )
                    nc.vector.tensor_copy(vt[:, mt, c * P : (c + 1) * P], tp[:])

            def emit_outB(w, ets, outp):
                """Out^T accumulation consuming unit w's e^T tile."""
                mmi = w % 8
                et = ets[w]
                for j in range(2):
                    m = 2 * mmi + j
                    for nt in range(4):
                        nc.tensor.matmul(
                            outp[nt // 2][:, (nt % 2) * C : (nt % 2) * C + C],
                            lhsT=et[:, j, nt * P : (nt + 1) * P],
                            rhs=vt[:, m, :],
                            start=False,
                            stop=(m == NT - 1),
                            skip_group_check=True,
                        )

            def emit_flush(quarter, outp):
                """Normalize + store the 4 finished n-tiles of a quarter."""
                for nt in range(4):
                    g = quarter * 4 + nt
                    osb = osb_pool.tile([P, C], F32, tag="osb", name="osb")
                    nc.vector.tensor_scalar_mul(
                        osb[:],
                        outp[nt // 2][:, (nt % 2) * C : (nt % 2) * C + C],
                        recip[:, g : g + 1],
                    )
                    nc.sync.dma_start(ot_d[g * P : (g + 1) * P, :], osb[:])

            ets = {}
            with (
                tc.tile_pool(name="psBs", bufs=1, space="PSUM") as psBs,
                tc.tile_pool(name="psBo", bufs=2, space="PSUM") as psBo,
            ):
                # psA opens AFTER psBo so it tops the pool stack and can be
                # released (LIFO) before the pipeline tail needs its banks
                psA_cm = tc.tile_pool(name="psA", bufs=1, space="PSUM")
                psA = psA_cm.__enter__()
                # A-block schedule: one per even unit, then densified at
                # units 24..27 so psA can close before the pipeline tail
                A_at = {2 * i: i for i in range(12)}
                for i in range(12, NT):
                    A_at[12 + i] = i

                # units 0..7: quarter 0 front + v transposes (the transpose
                # scratch shares psBo's bank-padded slots via tag="outp")
                for u in range(LAG):
                    ets[u] = emit_stB(u)
                    if u in A_at:
                        emit_A(A_at[u])
                    if u >= 4:
                        for mt in range(4 * (u - 4), 4 * (u - 3)):
                            emit_vt(mt)

                def alloc_outp(pool):
                    outp = [
                        pool.tile([P, 512], F32, tag="outp", name="outp")
                        for _ in range(2)
                    ]
                    for t in outp:
                        nc.tensor.matmul(
                            t[:],
                            lhsT=zeros[:],
                            rhs=ks[0][:, 0:512],
                            start=True,
                            stop=False,
                            skip_group_check=True,
                        )
                    return outp

                outp = None
                for u in range(LAG, 28):
                    w = u - LAG  # 0..19: quarters 0..2 at lag 8
                    if w % 8 == 0:
                        outp = alloc_outp(psBo)
                    ets[u] = emit_stB(u)
                    if u in A_at:
                        emit_A(A_at[u])
                    emit_outB(w, ets, outp)
                    del ets[w]
                    if w % 8 == 7:
                        emit_flush(w // 8, outp)
                # all 16 A blocks emitted; free psA's 4 banks for the tail
                psA_cm.__exit__(None, None, None)

                with tc.tile_pool(name="psBo2", bufs=2, space="PSUM") as psBo2:
                    outp3 = None
                    for u in range(28, 36):
                        if u == 28:
                            outp3 = alloc_outp(psBo2)
                        if u < NU:
                            ets[u] = emit_stB(u)
                        if u <= 31:
                            # quarter 2 finishes at lag 8
                            w = u - LAG
                            emit_outB(w, ets, outp)
                            del ets[w]
                            if w % 8 == 7:
                                emit_flush(2, outp)
                        # quarter 3 runs at lag 4
                        w3 = u - 4
                        emit_outB(w3, ets, outp3)
                        del ets[w3]
                        if w3 == NU - 1:
                            emit_flush(3, outp3)

    nc.compile()
    return nc


_NC_CACHE = None


def _get_program():
    global _NC_CACHE
    if _NC_CACHE is None:
        _NC_CACHE = build_program()
    return _NC_CACHE


def kernel(query, key, value, mask=0, _collect=None):
    """Full inputs in, full outputs out. Shards batch across 8 cores."""
    query = np.ascontiguousarray(np.asarray(query, dtype=np.float32))
    key = np.ascontiguousarray(np.asarray(key, dtype=np.float32))
    value = np.ascontiguousarray(np.asarray(value, dtype=np.float32))
    assert query.shape == (B, C, N), query.shape

    nc = _get_program()
    in_maps = [
        {"q": query[b], "k": key[b], "v": value[b]} for b in range(B)
    ]
    res = run_bass_kernel_spmd(nc, in_maps, core_ids=list(range(B)))
    if _collect is not None:
        _collect.append(res)
    p_attn = np.stack([res.results[b]["p"] for b in range(B)])
    out = np.stack([res.results[b]["ot"].T for b in range(B)])
    return out, p_attn
